# revision 81
# baseline (speedup 1.0000x reference)
"""Self-contained Trainium2 Bass kernel for the 3-layer GCN (AgriGraphGCN).

kernel(**inputs) -> (100000, 1) float32 risk scores, computed SPMD on 8
NeuronCores.

Strategy: nodes sharded by owner core.
- L1 (6-dim input) and L3 (scalar messages) run in PUSH mode: each core
  gathers its OWN nodes' messages from a local table (no collective
  dependency), scatters them across the FULL destination space via one-hot
  PE matmuls (narrow PSUM accumulators fit), and the per-half partial sums
  are combined with small ReduceScatters overlapped with the scatter work.
- L2 (128-wide messages) runs in PULL mode: build a bf16 table (project +
  PE-transpose + degree scaling) in 2 pieces, AllGather each piece as soon
  as written, dma_gather per-edge source rows, one-hot scatter into PSUM.
BatchNorm stats come from an AllReduce; the affine+relu application is
fused into the next layer's projection pass.
"""
import sys
sys.path.insert(0, "/opt/trn_rl_repo")

import numpy as np
import ml_dtypes


def make_cfg(full=True):
    if full:
        return dict(N=100000, E=640000, NPC=12500, NL=12544, G=14, IN=6,
                    PIECES=[28, 28, 28, 14], NHALF=7)
    return dict(N=4096, E=16384, NPC=512, NL=512, G=2, IN=6,
                PIECES=[2, 2], NHALF=2)


C = 8           # cores
H = 128         # hidden
NI_MAX = 1024   # dma_gather per-call limit (2048/4096 fail at runtime)
HWIN = 32768    # int16 index window (rows per gather source view)
BN_EPS = 1e-5


def host_prep(cfg, edge_index):
    """Common (cross-core) batch/op structure + per-core index data.

    All three layers share one PULL structure: edges partitioned by
    (dst-owner, dst-tile group, src piece, idx-half), sorted by local dst."""
    N, NPC, NL, G = cfg["N"], cfg["NPC"], cfg["NL"], cfg["G"]
    PIECES = cfg["PIECES"]
    NHALF = cfg["NHALF"]
    NP = len(PIECES)
    TILES = NL // 128
    assert sum(PIECES) == TILES
    assert NL % NHALF == 0 and (NL // NHALF) % 128 == 0
    HR = NL // NHALF               # local rows per half
    pstart = np.cumsum([0] + PIECES)
    NGRP = (TILES + G - 1) // G

    src = np.asarray(edge_index[0], dtype=np.int64)
    dst = np.asarray(edge_index[1], dtype=np.int64)
    deg = np.bincount(dst, minlength=N).astype(np.float32) + 1.0

    owner = dst // NPC
    dst_loc = (dst - owner * NPC).astype(np.int64)
    sowner = src // NPC
    sloc = (src - sowner * NPC).astype(np.int64)

    # ---------------- PULL structure (L2) ----------------
    stile = sloc // 128
    piece = np.searchsorted(pstart[1:-1], stile, side="right")
    prow = sowner * (np.array(PIECES)[piece] * 128) + (sloc - pstart[piece] * 128)
    half = prow // HWIN
    hidx = prow - half * HWIN
    assert hidx.max() < HWIN
    dtile = dst_loc // 128
    grp = dtile // G
    NH = [max(1, (C * PIECES[p] * 128 + HWIN - 1) // HWIN) for p in range(NP)]
    PORDER = list(range(NP))[::-1]      # later pieces first: their AG
    # section 1: ALL groups' first-AG'd-piece cells (gated only on the first
    # AllGather; partials evacuated to SBUF). section 2: group-outer over the
    # remaining pieces (kept in PSUM per group; finishes spread per group).
    p1st = PORDER[0]
    cells = ([(g, p1st, h) for g in range(NGRP) for h in range(NH[p1st])] +
             [(g, p, h) for g in range(NGRP) for p in PORDER[1:]
              for h in range(NH[p])])

    seg_edges = {}
    for c in range(C):
        m_c = owner == c
        for (g, p, h) in cells:
            m = m_c & (grp == g) & (piece == p) & (half == h)
            idx = np.nonzero(m)[0]
            order = np.argsort(dst_loc[idx], kind="stable")
            seg_edges[(c, g, p, h)] = idx[order]

    def build_batches(cells, seg, tile_of, base_tile):
        """Shared batch/op builder. seg[(c,)+cell] = sorted edge ids."""
        batches, ops = [], []
        for cell in cells:
            S = max(len(seg[(c,) + cell]) for c in range(C))
            S = ((S + 127) // 128) * 128
            if S == 0:
                continue
            nsub = S // 128
            lo = np.full(nsub, 10 ** 9, dtype=np.int64)
            hi = np.full(nsub, -1, dtype=np.int64)
            for c in range(C):
                idx = seg[(c,) + cell]
                if len(idx) == 0:
                    continue
                dt = tile_of[idx]
                for s in range((len(idx) + 127) // 128):
                    a, b = s * 128, min((s + 1) * 128, len(idx))
                    lo[s] = min(lo[s], dt[a:b].min())
                    hi[s] = max(hi[s], dt[a:b].max())
            off = 0
            while off < S:
                size = min(NI_MAX, S - off)
                b_id = len(batches)
                batches.append(cell + (size, off))
                for col in range(size // 128):
                    s = (off // 128) + col
                    if hi[s] < 0:
                        lo[s] = hi[s] = base_tile(cell)
                    for t in range(int(lo[s]), int(hi[s]) + 1):
                        ops.append((b_id, col, t))
                off += size
        return batches, ops

    batches, ops = build_batches(cells, seg_edges, dtile,
                                 lambda cell: cell[0] * G)
    # guarantee every tile gets at least one op (finish_tile must fire)
    covered = set(t for (_, _, t) in ops)
    for t in range(TILES):
        if t not in covered:
            g = t // G
            bsel = max(b_id for b_id, bt in enumerate(batches) if bt[0] == g)
            ops.append((bsel, 0, t))
    ops.sort(key=lambda o: (o[0], o[1]))
    NBAT, NOPS = len(batches), len(ops)
    last_op = {}
    for i, (_, _, t) in enumerate(ops):
        last_op[t] = i

    def wrap16_rep(vals, ncols):
        a = np.zeros(16 * ncols, dtype=np.int16)
        a[: len(vals)] = vals
        w = a.reshape(ncols, 16).T
        return np.tile(w, (8, 1))  # (128, ncols)

    def pack_inputs(batches, seg, idx_of):
        col_off, acc = [], 0
        for bt in batches:
            col_off.append(acc)
            acc += bt[-2] // 16
        gixd = np.zeros((C, 128, acc), dtype=np.int16)
        for c in range(C):
            for b_id, bt in enumerate(batches):
                cell, size, off = bt[:-2], bt[-2], bt[-1]
                idx = seg[(c,) + cell]
                pos = idx[off: off + size]
                gi = np.zeros(size, dtype=np.int16)
                gi[: len(pos)] = idx_of[pos].astype(np.int16)
                co = col_off[b_id]
                gixd[c, :, co: co + size // 16] = wrap16_rep(gi, size // 16)
        return col_off, acc, gixd

    def pack_dloc(batches, ops, seg, row_of):
        dl = np.full((C, 128, len(ops)), -1000.0, dtype=np.float32)
        for c in range(C):
            for o_id, (b_id, col, t) in enumerate(ops):
                bt = batches[b_id]
                cell, size, off = bt[:-2], bt[-2], bt[-1]
                idx = seg[(c,) + cell]
                a = off + col * 128
                pos = idx[a: a + 128]
                if len(pos):
                    v = row_of[pos].astype(np.float32) - t * 128.0
                    dl[c, : len(pos), o_id] = v
        return dl

    col_off, GIDX_COLS, gidx_data = pack_inputs(batches, seg_edges, hidx)
    dloc_data = pack_dloc(batches, ops, seg_edges, dst_loc)

    return dict(
        deg=deg,
        batches=batches, ops=ops, last_op=last_op,
        gidx_data=gidx_data, dloc_data=dloc_data, col_off=col_off,
        GIDX_COLS=GIDX_COLS,
        pstart=pstart, NH=NH, HR=HR,
        TILES=TILES, NGRP=NGRP, NP=NP, NBAT=NBAT, NOPS=NOPS,
    )


def build_graph(cfg, prep, params, num_msg_bufs=20):
    """Build the SPMD Bass graph. params: numpy dict (W1,W2,W3,b3,g1,be1,g2,be2)."""
    import sys
    sys.path.insert(0, "/opt/trn_rl_repo")
    from concourse import bacc, tile
    import concourse.mybir as mybir

    N, NPC, NL, G, IN = (cfg[k] for k in ["N", "NPC", "NL", "G", "IN"])
    PIECES = cfg["PIECES"]
    TILES, NGRP, NP = prep["TILES"], prep["NGRP"], prep["NP"]
    NBAT, NOPS = prep["NBAT"], prep["NOPS"]
    batches, ops = prep["batches"], prep["ops"]
    col_off, GIDX_COLS = prep["col_off"], prep["GIDX_COLS"]
    last_op = prep["last_op"]
    pstart = prep["pstart"]
    f32, bf16, i16 = mybir.dt.float32, mybir.dt.bfloat16, mybir.dt.int16
    TW = [H, H, H]              # table row width per layer
    TD = [bf16, bf16, bf16]     # table dtype per layer

    nc = bacc.Bacc("TRN2", target_bir_lowering=False, num_swdge_queues=4)

    # ---- DRAM I/O ----
    xs_d = nc.dram_tensor("xs", [128, TILES * IN], bf16, kind="ExternalInput")
    tbl1p_d = [nc.dram_tensor(f"tbl1p{p}", [C * PIECES[p] * 128, H], bf16,
                              kind="ExternalInput") for p in range(NP)]
    degc_d = nc.dram_tensor("degc", [128, TILES], f32, kind="ExternalInput")
    maskc_d = nc.dram_tensor("maskc", [128, TILES], bf16, kind="ExternalInput")
    gidx_d = nc.dram_tensor("gidx", [128, GIDX_COLS], i16, kind="ExternalInput")
    dloc_d = nc.dram_tensor("dloc", [128, NOPS], bf16, kind="ExternalInput")
    W1_d = nc.dram_tensor("W1", [IN, H], bf16, kind="ExternalInput")
    W2_d = nc.dram_tensor("W2", [H, H], bf16, kind="ExternalInput")
    W3_d = nc.dram_tensor("W3", [H, 1], bf16, kind="ExternalInput")
    gbe_d = nc.dram_tensor("gbe", [128, 4], f32, kind="ExternalInput")
    iden_d = nc.dram_tensor("iden", [128, 128], bf16, kind="ExternalInput")
    iota_d = nc.dram_tensor("iota", [128, 128], bf16, kind="ExternalInput")
    onesr_d = nc.dram_tensor("onesr", [1, 128], bf16, kind="ExternalInput")
    out_d = nc.dram_tensor("out", [128, TILES], f32, kind="ExternalOutput")

    tbl_loc = [[nc.dram_tensor(f"tbl{L}p{p}_loc", [PIECES[p] * 128, TW[L]],
                                TD[L]) for p in range(NP)] for L in range(3)]
    tbl_full = [[nc.dram_tensor(f"tbl{L}p{p}_full",
                                [C * PIECES[p] * 128, TW[L]],
                                TD[L], addr_space="Shared")
                 for p in range(NP)] for L in range(3)]
    st_in = [nc.dram_tensor(f"st{L}_in", [128, 2], f32) for L in range(2)]
    st_out = [nc.dram_tensor(f"st{L}_out", [128, 2], f32, addr_space="Shared")
              for L in range(2)]
    prime_in = nc.dram_tensor("prime_in", [1, 16], f32)
    prime_out = nc.dram_tensor("prime_out", [C, 16], f32, addr_space="Shared")

    b3 = float(params["b3"][0])
    rg = [list(range(C))]
    PORDER = list(range(NP))[::-1]

    chunks = []   # (piece, col_a, col_b); later pieces first (AG overlap)
    for p in list(range(NP))[::-1]:
        a, b = pstart[p] * 128, pstart[p + 1] * 128
        j = a
        while j < b:
            w = min(512, b - j)
            chunks.append((p, j, j + w))
            j += w
    NSL = len(chunks)

    from contextlib import ExitStack
    with tile.TileContext(nc) as tc, ExitStack() as ctx:
        res = ctx.enter_context(tc.tile_pool(name="res", bufs=1))
        mtp = ctx.enter_context(tc.tile_pool(name="mtp", bufs=3))
        stg = ctx.enter_context(tc.tile_pool(name="stg", bufs=3))
        tbp = ctx.enter_context(tc.tile_pool(name="tbp", bufs=4))
        msg = ctx.enter_context(tc.tile_pool(name="msg", bufs=num_msg_bufs))
        ohp = ctx.enter_context(tc.tile_pool(name="ohp", bufs=4))
        hpp = ctx.enter_context(tc.tile_pool(name="hpp", bufs=6))
        sqp = ctx.enter_context(tc.tile_pool(name="sqp", bufs=4))
        colp = ctx.enter_context(tc.tile_pool(name="colp", bufs=4))
        htp = ctx.enter_context(tc.tile_pool(name="htp", bufs=3))
        psB = ctx.enter_context(tc.tile_pool(name="psB", bufs=2, space="PSUM"))
        psACC = ctx.enter_context(tc.tile_pool(name="psACC", bufs=1,
                                               space="PSUM"))
        psST = ctx.enter_context(tc.tile_pool(name="psST", bufs=1,
                                              space="PSUM"))
        if True:
            # ---- residents ----
            xs = res.tile([128, TILES * IN], bf16, tag="xs")
            nc.sync.dma_start(out=xs[:], in_=xs_d[:, :])
            gidx = res.tile([128, GIDX_COLS], i16, tag="gidx")
            nc.sync.dma_start(out=gidx[:], in_=gidx_d[:, :])
            dloc = res.tile([128, NOPS], bf16, tag="dloc")
            nc.sync.dma_start(out=dloc[:], in_=dloc_d[:, :])
            W1 = res.tile([IN, H], bf16, tag="W1")
            nc.sync.dma_start(out=W1[:], in_=W1_d[:, :])
            W2 = res.tile([H, H], bf16, tag="W2")
            nc.sync.dma_start(out=W2[:], in_=W2_d[:, :])
            W3 = res.tile([H, 1], bf16, tag="W3")
            nc.sync.dma_start(out=W3[:], in_=W3_d[:, :])
            gbe = res.tile([128, 4], f32, tag="gbe")
            nc.sync.dma_start(out=gbe[:], in_=gbe_d[:, :])
            iden = res.tile([128, 128], bf16, tag="iden")
            nc.sync.dma_start(out=iden[:], in_=iden_d[:, :])
            iota = res.tile([128, 128], bf16, tag="iota")
            nc.sync.dma_start(out=iota[:], in_=iota_d[:, :])
            onesr = res.tile([1, 128], bf16, tag="onesr")
            nc.sync.dma_start(out=onesr[:], in_=onesr_d[:, :])
            maskc = res.tile([128, TILES], bf16, tag="maskc")
            nc.sync.dma_start(out=maskc[:], in_=maskc_d[:, :])
            degc = res.tile([128, TILES], f32, tag="degc")
            nc.sync.dma_start(out=degc[:], in_=degc_d[:, :])

            disc = res.tile([128, TILES], f32, tag="disc")
            nc.vector.reciprocal(out=disc[:], in_=degc[:])
            nc.scalar.sqrt(out=disc[:], in_=disc[:])
            # zero padded-node rows so aggregates/stats need no masking
            nc.vector.tensor_tensor(out=disc[:], in0=disc[:], in1=maskc[:],
                                    op=mybir.AluOpType.mult)

            # warm the collective path while CD0 runs
            nc.gpsimd.collective_compute(
                "AllGather", mybir.AluOpType.bypass,
                ins=[prime_in[:]], outs=[prime_out[:]],
                replica_groups=rg)

            # SBUF stashes of the LOCAL table rows (replace DRAM re-reads in
            # finish_tile: saves ~37K SDMA descriptors of read-back traffic)
            htloc_t = [res.tile([128, 128], bf16, tag=f"htloc{t}",
                                name=f"htloc{t}") for t in range(TILES)]
            scol = res.tile([128, TILES], bf16, tag="scol")
            ones6 = res.tile([IN, 1], bf16, tag="ones6")
            nc.vector.memset(ones6[:], 1.0)
            # SBUF accumulators for non-final-piece partial aggregates
            aggsb = [res.tile([128, TILES, fw], bf16, tag=f"aggsb{L}",
                              name=f"aggsb{L}")
                     for L, fw in enumerate([IN, H, 1])]
            for a_ in aggsb:
                nc.vector.memset(a_[:], 0.0)

            # pre-BN activations, tiled per A-phase chunk so the BN+relu
            # apply is a single wide ACT per chunk
            hpre_c = [res.tile([128, b - a], bf16, tag=f"hpre{j}",
                               name=f"hpre{j}")
                      for j, (p, a, b) in enumerate(chunks)]
            tile_chunk = {}
            for j, (p, a, b) in enumerate(chunks):
                for t in range(a // 128, b // 128):
                    tile_chunk[t] = (j, t - a // 128)

            def hpre_slice(t):
                j, r = tile_chunk[t]
                return hpre_c[j][:, r * 128:(r + 1) * 128]
            o_sb = res.tile([128, TILES], f32, tag="o_sb")
            MAXCNT = max(
                sum(1 for o in ops if o[0] == b) for b in range(NBAT))
            iorep = res.tile([128, MAXCNT, 128], bf16, tag="iorep")
            nc.vector.tensor_copy(
                out=iorep[:],
                in_=iota[:].rearrange("p (o f) -> p o f", o=1).broadcast_to(
                    [128, MAXCNT, 128]))

            def piece_of_tile(t):
                for p in range(NP):
                    if t < pstart[p + 1]:
                        return p
                raise AssertionError

            def ag_piece(L, p, hi=False):
                from contextlib import nullcontext
                with tc.high_priority() if hi else nullcontext():
                    nc.gpsimd.collective_compute(
                        "AllGather", mybir.AluOpType.bypass,
                        ins=[tbl_loc[L][p][:]], outs=[tbl_full[L][p][:]],
                        replica_groups=rg)

            # ---- phase A for L1: host-built dis*x table, copied into the
            # shared gather tables (no projection, no AllGather) ----
            xsv = xs[:].rearrange("p (t c) -> p t c", c=IN)

            def phase_A0():
                pass   # L1 gathers read the host-built input tables directly

            def emit_A_piece(L, Wt, AB, pp):
                """BN-apply + project + transpose + dis-scale -> table,
                for the chunks of piece pp only."""
                for j, (p, a, b) in enumerate(chunks):
                    if p != pp:
                        continue
                    nt = (b - a) // 128
                    hc = htp.tile([128, 512], bf16, tag="hc")
                    nc.scalar.activation(
                        hc[:, : b - a], hpre_c[j][:],
                        mybir.ActivationFunctionType.Relu,
                        scale=AB[:, 0:1], bias=AB[:, 1:2])
                    if L == 1:
                        pa = psST.tile([128, 512], f32, tag="pa")
                        nc.tensor.matmul(pa[:, : b - a], Wt[:],
                                         hc[:, : b - a],
                                         start=True, stop=True)
                        mt = mtp.tile([128, 512], bf16, tag="mt")
                        nc.vector.tensor_copy(out=mt[:, : b - a],
                                              in_=pa[:, : b - a])
                        for jj in range(nt):
                            t = (a // 128) + jj
                            pb = psB.tile([128, 128], f32, tag="psB")
                            nc.tensor.matmul(pb[:],
                                             mt[:, jj * 128:(jj + 1) * 128],
                                             iden[:], start=True, stop=True)
                            # stash local table rows in SBUF (selfloop reuse)
                            nc.vector.tensor_scalar_mul(
                                out=htloc_t[t][:], in0=pb[:],
                                scalar1=disc[:, t: t + 1])
                            tt = t - pstart[p]
                            nc.sync.dma_start(
                                out=tbl_loc[L][p][tt * 128:(tt + 1) * 128, :],
                                in_=htloc_t[t][:])
                    else:
                        sg = stg.tile([128, 4, TW[L]], TD[L], tag="stg")
                        pa = psST.tile([1, 512], f32, tag="pa")
                        nc.tensor.matmul(pa[:, : b - a], Wt[:],
                                         hc[:, : b - a],
                                         start=True, stop=True)
                        m3c = mtp.tile([1, 512], bf16, tag="m3c")
                        nc.vector.tensor_copy(out=m3c[:, : b - a],
                                              in_=pa[:, : b - a])
                        for jj in range(nt):
                            t = (a // 128) + jj
                            pb = psB.tile([128, 128], f32, tag="psB")
                            nc.tensor.matmul(pb[:],
                                             m3c[:, jj * 128:(jj + 1) * 128],
                                             onesr[:], start=True, stop=True)
                            nc.vector.tensor_scalar_mul(
                                out=sg[:, jj, :], in0=pb[:],
                                scalar1=disc[:, t: t + 1])
                            # stash s column (selfloop reuse in CD2 finish)
                            nc.vector.tensor_scalar_mul(
                                out=scol[:, t: t + 1], in0=pb[:, 0:1],
                                scalar1=disc[:, t: t + 1])
                        tloc0 = (a // 128) - pstart[p]
                        out_ap = tbl_loc[L][p][tloc0 * 128: tloc0 * 128 + nt * 128, :]
                        nc.sync.dma_start(
                            out=out_ap.rearrange("(j q) h -> q j h", q=128),
                            in_=sg[:, :nt, :])

            # ---- CD (pull), with per-piece table-build interleaved ----
            def phase_CD(L, Wt=None, AB=None):
                FW = IN if L == 0 else (H if L == 1 else 1)
                ops_span = {}
                for o_id, (bb, _, _) in enumerate(ops):
                    if bb not in ops_span:
                        ops_span[bb] = [o_id, 0]
                    ops_span[bb][1] += 1
                p1st = PORDER[0]

                def close_key(key, pst):
                    kind, g = key[0], key[1]
                    ntl = min(G, TILES - g * G)
                    if kind == 's1':
                        # first piece's partial: accumulate into SBUF
                        nc.vector.tensor_tensor(
                            out=aggsb[L][:, g * G: g * G + ntl, :],
                            in0=aggsb[L][:, g * G: g * G + ntl, :],
                            in1=pst[:, :ntl, :], op=mybir.AluOpType.add)
                    else:
                        # remaining pieces accumulated in psum; fold the SBUF
                        # partial + selfloop per tile of the group
                        for t in range(g * G, g * G + ntl):
                            finish_tile(L, t, pst, t - g * G, sts)
                s2_start = next((bb for bb in range(NBAT)
                                 if batches[bb][1] != p1st), NBAT)
                trig = {}
                if L > 0:
                    # table-build COMPUTE for all pieces up front (gated only
                    # on AB; runs on ACT/PE under section-1's gathers). The
                    # AG triggers for pieces 2..4 are interspersed through
                    # section 1 so they never stall the Pool queue.
                    emit_A_piece(L, Wt, AB, p1st)
                    ag_piece(L, p1st, hi=True)
                    for pp in PORDER[1:]:
                        emit_A_piece(L, Wt, AB, pp)
                    k = len(PORDER) - 1
                    for i, pp in enumerate(PORDER[1:]):
                        pos = min((i + 2) * s2_start // (k + 1),
                                  s2_start)
                        trig.setdefault(pos, []).append(pp)
                # stats accumulators allocated AFTER the A-emits so the
                # psST/psA pools rotate cleanly (sequential lifetimes)
                sts = None
                if L == 0:
                    # 6-d moments: [?, 0]=first, [?, 1:7]=second-moment matrix
                    sts = psST.tile([IN, 1 + IN], f32, tag="stat")
                    nc.vector.memset(sts[:], 0.0)
                if L == 1:
                    # col 0: first moment; cols 1:129: second-moment matrix
                    sts = psST.tile([128, 1 + H], f32, tag="stat")
                    nc.vector.memset(sts[:], 0.0)
                cur_key = None
                pst = None
                for bb in range(NBAT):
                    for pp in trig.get(bb, ()):
                        ag_piece(L, pp)
                    g, p, h, size, off = batches[bb]
                    key = ('s1', g, h) if p == p1st else ('s2', g)
                    if key != cur_key:
                        if cur_key is not None:
                            close_key(cur_key, pst)
                        cur_key = key
                        pst = psACC.tile([128, G, FW], f32, tag="acc",
                                         name=f"accL{L}{key[0]}g{g}")
                        nc.vector.memset(pst[:], 0.0)
                    m = msg.tile([128, NI_MAX // 128, TW[L]], TD[L],
                                 tag="msg")
                    hoff = h * HWIN
                    hrows = min(HWIN, C * PIECES[p] * 128 - hoff)
                    src_t = tbl1p_d[p] if L == 0 else tbl_full[L][p]
                    nc.gpsimd.dma_gather(
                        out_ap=m[:, : size // 128, :],
                        in_ap=src_t[hoff: hoff + hrows, :],
                        idxs_ap=gidx[:, col_off[bb]: col_off[bb] + size // 16],
                        num_idxs=size, num_idxs_reg=size, elem_size=TW[L],
                        queue_num=bb % 4)
                    o0, cnt = ops_span[bb]
                    oh = ohp.tile([128, cnt, 128], TD[L], tag="oh",
                                  name=f"ohL{L}b{bb}")
                    dl_b = dloc[:, o0: o0 + cnt].rearrange(
                        "p (o f) -> p o f", f=1).broadcast_to([128, cnt, 128])
                    nc.vector.tensor_tensor(out=oh[:],
                                            in0=iorep[:, : cnt, :],
                                            in1=dl_b,
                                            op=mybir.AluOpType.is_equal)
                    for j in range(cnt):
                        o_id = o0 + j
                        _, col, t = ops[o_id]
                        ti = t - g * G
                        nc.tensor.matmul(pst[:, ti, :], oh[:, j, :],
                                         m[:, col, 0:FW],
                                         start=False, stop=False,
                                         skip_group_check=True)
                close_key(cur_key, pst)
                if L == 0:
                    return finish_layer_stats(0, sts)
                if L == 1:
                    return finish_layer_stats(1, sts)
                nc.sync.dma_start(out=out_d[:, :], in_=o_sb[:])
                return None

            def finish_tile(L, t, pst, ti, sts):
                p = piece_of_tile(t)
                tt = t - pstart[p]
                # fold in the SBUF partial (pieces other than the final one)
                nc.tensor.matmul(pst[:, ti, :], iden[:], aggsb[L][:, t, :],
                                 start=False, stop=False, skip_group_check=True)
                if L == 0:
                    # self-loop on 6-wide aggregate, then project by W1
                    nc.tensor.matmul(pst[:, ti, :], iden[:], xsv[:, t, :],
                                     start=False, stop=False,
                                     skip_group_check=True)
                    hp6 = hpp.tile([128, IN], bf16, tag="hp6")
                    nc.scalar.activation(hp6[:], pst[:, ti, :],
                                         mybir.ActivationFunctionType.Copy,
                                         scale=disc[:, t: t + 1])
                    # 6-d moment accumulation (stats projected once at the
                    # end): m1 += hp6^T mask, M2 += hp6^T hp6
                    nc.tensor.matmul(sts[:, 0:1], hp6[:], maskc[:, t: t + 1],
                                     start=False, stop=False,
                                     skip_group_check=True)
                    nc.tensor.matmul(sts[:, 1:1 + IN], hp6[:], hp6[:],
                                     start=False, stop=False,
                                     skip_group_check=True)
                    pb6 = psB.tile([IN, 128], f32, tag="psB", name=f"pb6t{t}")
                    nc.tensor.matmul(pb6[:], hp6[:], iden[:],
                                     start=True, stop=True)
                    c6 = sqp.tile([IN, 128], bf16, tag="c6")
                    nc.scalar.activation(c6[:], pb6[:],
                                         mybir.ActivationFunctionType.Copy)
                    pbF = psB.tile([128, 128], f32, tag="psB", name=f"pbFt{t}")
                    nc.tensor.matmul(pbF[:], W1[:], c6[:],
                                     start=True, stop=True)
                    nc.scalar.activation(hpre_slice(t), pbF[:],
                                         mybir.ActivationFunctionType.Copy)
                    return
                if L == 2:
                    nc.tensor.matmul(pst[:, ti, :], iden[:],
                                     scol[:, t: t + 1],
                                     start=False, stop=False,
                                     skip_group_check=True)
                    nc.scalar.activation(o_sb[:, t: t + 1], pst[:, ti, 0:1],
                                         mybir.ActivationFunctionType.Sigmoid,
                                         scale=disc[:, t: t + 1], bias=b3)
                    return
                nc.tensor.matmul(pst[:, ti, :], iden[:], htloc_t[t][:],
                                 start=False, stop=False, skip_group_check=True)
                hp = hpp.tile([128, 128], bf16, tag="hp")
                nc.scalar.activation(hp[:], pst[:, ti, :],
                                     mybir.ActivationFunctionType.Copy,
                                     scale=disc[:, t: t + 1])
                nc.tensor.matmul(sts[:, 0:1], hp[:], maskc[:, t: t + 1],
                                 start=False, stop=False, skip_group_check=True)
                nc.tensor.matmul(sts[:, 1:1 + H], hp[:], hp[:],
                                 start=False, stop=False, skip_group_check=True)
                pb = psB.tile([128, 128], f32, tag="psB")
                nc.tensor.matmul(pb[:], hp[:], iden[:], start=True, stop=True)
                nc.scalar.activation(hpre_slice(t), pb[:],
                                     mybir.ActivationFunctionType.Copy)

            def finish_layer_stats(L, sts):
                stat = colp.tile([128, 2], f32, tag="stat")
                if L == 0:
                    # project 6-d moments through W1:
                    #   sum_f = W1^T m1 ;  sumsq_f = 1^T (W1 ⊙ (M2 W1))
                    stb = colp.tile([IN, 1 + IN], bf16, tag="stb")
                    nc.scalar.activation(stb[:], sts[:],
                                         mybir.ActivationFunctionType.Copy)
                    z1s = psB.tile([128, 1], f32, tag="psB", name="z1s")
                    nc.tensor.matmul(z1s[:], W1[:], stb[:, 0:1],
                                     start=True, stop=True)
                    nc.scalar.activation(stat[:, 0:1], z1s[:],
                                         mybir.ActivationFunctionType.Copy)
                    pP = psB.tile([IN, 128], f32, tag="psB", name="pP")
                    nc.tensor.matmul(pP[:], stb[:, 1:1 + IN], W1[:],
                                     start=True, stop=True)
                    wp = colp.tile([IN, 128], bf16, tag="wp")
                    nc.vector.tensor_tensor(out=wp[:], in0=W1[:], in1=pP[:],
                                            op=mybir.AluOpType.mult)
                    dps = psB.tile([128, 1], f32, tag="psB", name="dps")
                    nc.tensor.matmul(dps[:], wp[:], ones6[:],
                                     start=True, stop=True)
                    nc.scalar.activation(stat[:, 1:2], dps[:],
                                         mybir.ActivationFunctionType.Copy)
                else:
                    nc.vector.tensor_copy(out=stat[:, 0:1], in_=sts[:, 0:1])
                    m2d = sqp.tile([128, 128], f32, tag="m2d")
                    nc.vector.tensor_tensor(out=m2d[:], in0=sts[:, 1:1 + H],
                                            in1=iden[:],
                                            op=mybir.AluOpType.mult)
                    nc.vector.tensor_reduce(
                        out=stat[:, 1:2], in_=m2d[:],
                        axis=mybir.AxisListType.X, op=mybir.AluOpType.add)
                nc.sync.dma_start(out=st_in[L][:, :], in_=stat[:])
                with tc.high_priority():
                    nc.gpsimd.collective_compute(
                        "AllReduce", mybir.AluOpType.add,
                        ins=[st_in[L][:]], outs=[st_out[L][:]],
                        replica_groups=rg)
                stg_ = colp.tile([128, 2], f32, tag="statg")
                nc.sync.dma_start(out=stg_[:], in_=st_out[L][:, :])
                mu = colp.tile([128, 4], f32, tag="mu")
                inv_n = 1.0 / float(N)
                nc.vector.tensor_scalar_mul(out=mu[:, 0:2], in0=stg_[:],
                                            scalar1=inv_n)
                nc.vector.tensor_tensor(out=mu[:, 2:3], in0=mu[:, 0:1],
                                        in1=mu[:, 0:1],
                                        op=mybir.AluOpType.mult)
                nc.vector.tensor_tensor(out=mu[:, 2:3], in0=mu[:, 1:2],
                                        in1=mu[:, 2:3],
                                        op=mybir.AluOpType.subtract)
                nc.vector.tensor_scalar_add(out=mu[:, 2:3], in0=mu[:, 2:3],
                                            scalar1=BN_EPS)
                nc.vector.reciprocal(out=mu[:, 3:4], in_=mu[:, 2:3])
                nc.scalar.sqrt(out=mu[:, 3:4], in_=mu[:, 3:4])
                AB = colp.tile([128, 2], f32, tag=f"AB{L}", name=f"AB{L}")
                gcol = gbe[:, 2 * L: 2 * L + 1]
                becol = gbe[:, 2 * L + 1: 2 * L + 2]
                nc.vector.tensor_tensor(out=AB[:, 0:1], in0=gcol,
                                        in1=mu[:, 3:4],
                                        op=mybir.AluOpType.mult)
                nc.vector.tensor_tensor(out=AB[:, 1:2], in0=mu[:, 0:1],
                                        in1=AB[:, 0:1],
                                        op=mybir.AluOpType.mult)
                nc.vector.tensor_tensor(out=AB[:, 1:2], in0=becol,
                                        in1=AB[:, 1:2],
                                        op=mybir.AluOpType.subtract)
                return AB

            # ---- run 3 layers (A-phase interleaved per piece inside CD) ----
            phase_A0()
            AB0 = phase_CD(0)
            AB1 = phase_CD(1, W2, AB0)
            phase_CD(2, W3, AB1)

    nc.finalize()
    return nc


def make_inputs(cfg, prep, inputs, core):
    """Per-core input map."""
    N, NPC, NL, IN = cfg["N"], cfg["NPC"], cfg["NL"], cfg["IN"]
    TILES = NL // 128
    bf = ml_dtypes.bfloat16
    x = np.asarray(inputs["x"], np.float32)
    deg = prep["deg"]

    if "tbl1p" not in prep:
        # replicated full dis*x gather tables (elementwise input prep)
        disF = (1.0 / np.sqrt(prep["deg"])).astype(np.float32)
        xsF = disF[:, None] * x                      # (N, IN)
        pstart = prep["pstart"]
        tabs = []
        for p in range(len(pstart) - 1):
            rows_p = (pstart[p + 1] - pstart[p]) * 128
            arr = np.zeros((C * rows_p, 128), np.float32)
            for c in range(C):
                a = pstart[p] * 128
                b = min(pstart[p + 1] * 128, NPC)
                if b > a:
                    g0 = c * NPC + a
                    arr[c * rows_p: c * rows_p + (b - a), :IN] = \
                        xsF[g0: g0 + (b - a)]
            tabs.append(arr.astype(bf))
        prep["tbl1p"] = tabs

    xl = np.zeros((NL, IN), np.float32)
    xl[:NPC] = x[core * NPC:(core + 1) * NPC]
    degl = np.ones(NL, np.float32)
    degl[:NPC] = deg[core * NPC:(core + 1) * NPC]
    disl = 1.0 / np.sqrt(degl)
    mask = np.zeros(NL, np.float32)
    mask[:NPC] = 1.0
    # xs = dis*x, node-tiled [128, TILES*IN]
    xs = (disl[:, None] * xl).reshape(TILES, 128, IN)
    xs = xs.transpose(1, 0, 2).reshape(128, TILES * IN)

    gbe = np.stack([
        np.asarray(inputs["g1"], np.float32), np.asarray(inputs["be1"], np.float32),
        np.asarray(inputs["g2"], np.float32), np.asarray(inputs["be2"], np.float32),
    ], axis=1)  # (128, 4)

    return {
        "xs": xs.astype(bf),
        **{f"tbl1p{p}": t for p, t in enumerate(prep["tbl1p"])},
        "degc": degl.reshape(TILES, 128).T.copy(),
        "maskc": mask.reshape(TILES, 128).T.astype(bf).copy(),
        "gidx": prep["gidx_data"][core],
        "dloc": prep["dloc_data"][core].astype(bf),
        "W1": np.asarray(inputs["W1"], np.float32).astype(bf),
        "W2": np.asarray(inputs["W2"], np.float32).astype(bf),
        "W3": np.asarray(inputs["W3"], np.float32).astype(bf),
        "gbe": gbe,
        "iden": np.eye(128, dtype=np.float32).astype(bf),
        "iota": np.tile(np.arange(128, dtype=np.float32), (128, 1)).astype(bf),
        "onesr": np.ones((1, 128), np.float32).astype(bf),
    }


def unshard_output(cfg, results):
    N, NPC, NL = cfg["N"], cfg["NPC"], cfg["NL"]
    TILES = NL // 128
    out = np.zeros((N, 1), np.float32)
    for c in range(C):
        o = results[c]["out"]            # (128, TILES)
        flat = o.T.reshape(NL)           # node-major
        out[c * NPC:(c + 1) * NPC, 0] = flat[:NPC]
    return out


def _ensure_axon_hooks_shim():
    """bass_utils' trace path imports antenv.axon_hooks, which this image
    lacks; register a no-op so a stray BASS_TRACE=1 can't crash the run."""
    import types
    if 'antenv.axon_hooks' in sys.modules:
        return
    try:
        import antenv
        from antenv import axon_hooks  # noqa: F401
    except ImportError:
        mod = types.ModuleType('antenv.axon_hooks')
        _hook = [None]
        mod.set_axon_ntff_profile_hook = lambda h: _hook.__setitem__(0, h)
        mod.get_axon_ntff_profile_hook = lambda: _hook[0]
        sys.modules['antenv.axon_hooks'] = mod
        try:
            antenv.axon_hooks = mod
        except Exception:
            pass


def kernel(**inputs):
    import os
    import numpy as np
    from concourse import bass_utils

    _ensure_axon_hooks_shim()
    cfg = make_cfg(full=True)
    inputs = {k: np.asarray(v) for k, v in inputs.items()}
    prep = host_prep(cfg, inputs["edge_index"])
    nc = build_graph(cfg, prep, inputs)
    in_maps = [make_inputs(cfg, prep, inputs, c) for c in range(C)]
    prev = os.environ.get("BASS_NEVER_TRACE")
    os.environ["BASS_NEVER_TRACE"] = "1"
    try:
        res = bass_utils.run_bass_kernel_spmd(nc, in_maps, list(range(C)), trace=False)
    finally:
        if prev is None:
            os.environ.pop("BASS_NEVER_TRACE", None)
        else:
            os.environ["BASS_NEVER_TRACE"] = prev
    return unshard_output(cfg, [res.results[c] for c in range(C)])

